# revision 20
# baseline (speedup 1.0000x reference)
"""Two-layer GAT (single-head GATConv x2 + log_softmax) on 8 Trainium2 cores.

Edge-parallel, dst-sharded. Nodes split into 8 contiguous dst ranges; each
core owns the edges targeting its range and computes its output rows fully
locally. Host preprocessing (graph structure only) sorts edges by dst,
partitions them into 4 residue classes by (global table row % 4) so the fast
int16 dma_gather can address the whole AllGathered node table via a
stride-4 view, and cuts each class into 128-edge node-aligned tiles
(dst span <= 32).

Per layer: fp16 node-table rows (256B: h | a_src-score | pad) are gathered
by src with ONE batched dma_gather per residue per chunk (2048 descriptors
amortize the ~1us SWDGE fixed cost); per-dst a_dst scores are expanded per
edge by tiny PE matmuls against streamed indicator-transpose tiles; logits
get exp(leaky_relu(.) - B) on the scalar engine (B = global bound via
AllReduce-max; a common shift cancels in the softmax); messages and segment
sums ride one PE matmul per tile (lhsT = 0/1 indicator, rhs = w*[h|1]) into
dst-major PSUM slots which are copied to SBUF and scattered with ONE
batched int16 dma_scatter_add per residue per chunk (rows disjoint within a
residue by construction, onto pre-zeroed per-residue accumulators; unused
slots land on per-partition dump rows). The flush phase sums the 4
accumulators, divides by the segment sum, applies bias+ReLU (layer 1 ->
next table + AllGather) or bias+log_softmax (layer 2 -> output).

The program is identical on all 8 cores (SPMD); all per-core variation
lives in data tensors.
"""

import math
import numpy as np

import concourse.bass as bass
import concourse.mybir as mybir
from concourse import library_config
from concourse.bass import IndirectOffsetOnAxis
from concourse.tile import TileContext
from concourse.masks import make_identity

FP32 = mybir.dt.float32
FP16 = mybir.dt.float16
I32 = mybir.dt.int32
I16 = mybir.dt.int16

CORES = 8
TSPAN = 32      # max dst span of one edge tile
TEDGE = 128     # edges per tile
SEG_T = 16      # tiles per residue segment (= per psum group / scatter)
NRES = 4        # residue classes (table stride)
CH_T = SEG_T * NRES   # tiles per chunk = 64
CH_E = CH_T * TEDGE   # edges per chunk = 8192
GIDX = SEG_T * TEDGE  # gather indices per residue per chunk = 2048
SIDX = 6 * TEDGE      # scatter tokens per residue per chunk = 768
WIN = 512       # dst rows per flush window (h2ext matmul width)
B_MARGIN = 5.0  # exp shift: bias = MARGIN - B so w <= e^MARGIN
ROW = 128       # fp16 elements per table/acc row (256B)
ZCOL = 3232     # columns of the SBUF zero tile used for accumulator init


def wrap16(a):
    """[n] -> [128, n/16] int16 wrapped (i = s*16 + p%16) + replicated."""
    n = a.shape[0]
    assert n % 16 == 0
    w = a.reshape(n // 16, 16).T.astype(np.int16)      # [16, n/16]
    return np.tile(w, (8, 1))                          # [128, n/16]


# ---------------------------------------------------------------------------
# Host-side graph preprocessing (structure only)
# ---------------------------------------------------------------------------

def preprocess(edges_index: np.ndarray, n_nodes: int, n_cores: int = CORES):
    npc = int(math.ceil(n_nodes / n_cores))
    pad = int(math.ceil(npc / WIN)) * WIN
    gt = pad * n_cores
    assert gt // NRES <= 32767, "table too large for int16 residue gather"
    dump = pad  # scatter dump rows [pad, pad+128) (accum has pad+128 rows)

    src = np.concatenate([edges_index[0], np.arange(n_nodes, dtype=np.int64)])
    dst = np.concatenate([edges_index[1], np.arange(n_nodes, dtype=np.int64)])
    order = np.argsort(dst, kind="stable")
    src = src[order]
    dst = dst[order]
    tcore = src // npc
    g_row = (tcore * pad + (src - tcore * npc)).astype(np.int64)

    data_tiles = []
    nch = 1
    for c in range(n_cores):
        lo = np.searchsorted(dst, c * npc, "left")
        hi = np.searchsorted(dst, min((c + 1) * npc, n_nodes), "left")
        dloc = (dst[lo:hi] - c * npc).astype(np.int64)
        rows = g_row[lo:hi]
        res = rows % NRES
        perres = []
        for r in range(NRES):
            er = np.where(res == r)[0]
            dl = dloc[er]
            rw = rows[er]
            ne = len(er)
            s = 0
            pending = []   # (rows, dloc, off)
            while s < ne:
                d0 = dl[s]
                e_max = int(min(s + TEDGE,
                                np.searchsorted(dl, d0 + TSPAN, "left"), ne))
                # node alignment: never split a dst segment across tiles
                if e_max < ne and dl[e_max] == dl[e_max - 1]:
                    e_cut = int(np.searchsorted(dl, dl[e_max], "left"))
                    assert e_cut > s, "degree too large for tile"
                    e_max = e_cut
                pending.append((rw[s:e_max], dl[s:e_max], int(dl[s])))
                s = e_max
            perres.append(pending)
            nch = max(nch, int(math.ceil(len(pending) / SEG_T)))
        data_tiles.append(perres)

    data = []
    for c in range(n_cores):
        idx16 = np.zeros((nch, NRES, GIDX), np.int64)
        ind = np.zeros((nch, TEDGE, CH_T, TSPAN), np.float16)
        indt = np.zeros((nch, TEDGE, SEG_T, TEDGE), np.float16)
        adst_idx = np.zeros((nch, CH_T, 1), np.int32)
        # default: per-partition dump rows (spread the garbage writes of
        # unused slots over 128 distinct rows)
        scat16 = np.broadcast_to(dump + np.arange(128, dtype=np.int64),
                                 (nch, NRES, 6, 128)).copy()
        for r in range(NRES):
            pend = data_tiles[c][r]
            for t, (rw, dl, off) in enumerate(pend):
                ch, k = divmod(t, SEG_T)
                j = r * SEG_T + k
                kk = len(rw)
                d_rel = dl - off
                span = int(d_rel[-1]) + 1 if kk else 0
                idx16[ch, r, k * TEDGE:k * TEDGE + kk] = rw // NRES
                if kk:
                    ind[ch, np.arange(kk), j, d_rel] = 1.0
                    indt[ch, TSPAN * (j % 4) + d_rel, j // 4, np.arange(kk)] = 1.0
                adst_idx[ch, j, 0] = off
                # scatter token for tile k: column b=k//3, partition
                # 32*(k%3)+d
                if span:
                    b = k // 3
                    p0 = TSPAN * (k % 3)
                    scat16[ch, r, b, p0:p0 + span] = off + np.arange(span)
        # wrap16 per (chunk, residue) and stack residues along columns
        idxg = np.stack(
            [np.stack([wrap16(idx16[ch, r]) for r in range(NRES)], 1)
             for ch in range(nch)], 0)            # [nch, 128, NRES, GIDX/16]
        idxg = idxg.reshape(nch, 128, NRES * (GIDX // 16))
        sc = scat16.reshape(nch, NRES, SIDX)
        scg = np.stack(
            [np.stack([wrap16(sc[ch, r]) for r in range(NRES)], 1)
             for ch in range(nch)], 0)            # [nch, 128, NRES, SIDX/16]
        scg = scg.reshape(nch, 128, NRES * (SIDX // 16))
        data.append(dict(idxg=idxg, ind=ind, indt=indt, adst_idx=adst_idx,
                         scg=scg))
    return data, nch, npc, pad


# ---------------------------------------------------------------------------
# Device program
# ---------------------------------------------------------------------------

def split_excess_waits(nc, cap=1):
    """This walrus build accepts at most `cap` sync waits per instruction;
    split the extras onto preceding same-engine NOPs."""
    for fn in nc.m.functions:
        for blk in fn.blocks:
            lst = list(blk.instructions)
            changed = False
            i = 0
            while i < len(lst):
                inst = lst[i]
                si = inst.sync_info
                if si is not None and len(si.on_wait) > cap:
                    w = list(si.on_wait)
                    nop = mybir.InstNoOp(
                        name=nc.get_next_instruction_name(), engine=inst.engine,
                        sync_info=mybir.SyncInfo(on_wait=w[:cap], on_update=[]))
                    inst.sync_info = mybir.SyncInfo(
                        on_wait=w[cap:], on_update=list(si.on_update))
                    lst.insert(i, nop)
                    changed = True
                i += 1
            if changed:
                blk.instructions = lst


def build_nc(nch, pad, npc, d_in, d_hid, d_out, n_cores=CORES):
    assert pad % WIN == 0 and pad % 128 == 0
    nblk = pad // 128
    wins = pad // WIN
    gt = pad * n_cores
    M1 = d_hid + 1
    M2 = d_out + 1
    AluOp = mybir.AluOpType
    Act = mybir.ActivationFunctionType
    rg = [list(range(n_cores))]

    nc = bass.Bass()

    x = nc.dram_tensor("x", [pad, d_in], FP32, kind="ExternalInput")
    W1 = nc.dram_tensor("W1", [d_in, d_hid], FP32, kind="ExternalInput")
    a1 = nc.dram_tensor("a1", [d_hid, 2], FP32, kind="ExternalInput")
    b1 = nc.dram_tensor("b1", [1, d_hid], FP32, kind="ExternalInput")
    W2 = nc.dram_tensor("W2", [d_hid, d_out], FP32, kind="ExternalInput")
    a2 = nc.dram_tensor("a2", [d_out, 2], FP32, kind="ExternalInput")
    b2 = nc.dram_tensor("b2", [1, d_out], FP32, kind="ExternalInput")
    idxg_s = nc.dram_tensor("idxg_s", [nch, 128, NRES * (GIDX // 16)], I16,
                            kind="ExternalInput")
    ind_s = nc.dram_tensor("ind_s", [nch, TEDGE, CH_T, TSPAN], FP16, kind="ExternalInput")
    indt_s = nc.dram_tensor("indt_s", [nch, TEDGE, SEG_T, TEDGE], FP16, kind="ExternalInput")
    adst_i = nc.dram_tensor("adst_i", [nch, CH_T, 1], I32, kind="ExternalInput")
    scg_s = nc.dram_tensor("scg_s", [nch, 128, NRES * (SIDX // 16)], I16,
                           kind="ExternalInput")
    out = nc.dram_tensor("out", [pad, d_out], FP32, kind="ExternalOutput")

    with TileContext(nc) as tc:
        with (
            tc.tile_pool(name="const", bufs=1) as constp,
            tc.tile_pool(name="sb", bufs=2) as sb,
            tc.tile_pool(name="sb3", bufs=4) as sb3,
            tc.tile_pool(name="ps", bufs=4, space="PSUM") as ps,
            tc.tile_pool(name="psm", bufs=2, space="PSUM") as psm,
            tc.tile_pool(name="dram", bufs=1, space="DRAM") as dr,
        ):
            ident = constp.tile([128, 128], FP32)
            make_identity(nc, ident[:])
            identh = constp.tile([128, 128], FP16)
            nc.vector.tensor_copy(identh[:], ident[:])
            ones_row = constp.tile([1, 128], FP16)
            nc.gpsimd.memset(ones_row[:], 1.0)
            rrep = constp.tile([TSPAN, 128], FP16)
            for a in range(4):
                nc.vector.tensor_copy(rrep[:, TSPAN * a:TSPAN * (a + 1)],
                                      identh[:TSPAN, :TSPAN])
            qmask = constp.tile([128, 1, 4], FP16)
            nc.gpsimd.memset(qmask[:], 0.0)
            for a in range(4):
                nc.gpsimd.memset(qmask[TSPAN * a:TSPAN * (a + 1), 0:1, a:a + 1], 1.0)

            # tables are stored 4 node-rows per 512-wide line so the int16
            # dma_gather can address them through a stride-4 residue view
            tab1_sh = dr.tile([pad, ROW], FP16)
            tab1 = dr.tile([gt // NRES, NRES * ROW], FP16, addr_space="Shared")
            tab2_sh = dr.tile([pad, ROW], FP16)
            tab2 = dr.tile([gt // NRES, NRES * ROW], FP16, addr_space="Shared")
            adst1 = dr.tile([1, pad + TSPAN], FP16)
            adst2 = dr.tile([1, pad + TSPAN], FP16)
            accs1 = [dr.tile([pad + 128, ROW], FP16, name=f"acc1_{r}")
                     for r in range(NRES)]
            accs2 = [dr.tile([pad + 128, ROW], FP16, name=f"acc2_{r}")
                     for r in range(NRES)]
            bmax_sh = dr.tile([1, 2], FP32)
            bmax1 = dr.tile([1, 2], FP32, addr_space="Shared")
            bmax2 = dr.tile([1, 2], FP32, addr_space="Shared")

            # zero accumulators with a few big DMAs from a wide zero tile
            zbig = constp.tile([128, ZCOL], FP16)
            nc.gpsimd.memset(zbig[:], 0.0)
            # all standard-library gpsimd work (iota/memset) is above; the
            # extended dma_gather/dma_scatter_add kernels live in the mlp
            # DKL library
            nc.gpsimd.load_library(library_config.mlp)
            # shared count registers (a fresh to_reg per call site exhausts
            # the Pool register file at 648 call sites)
            reg_g = nc.gpsimd.to_reg(GIDX // 2)
            reg_s = nc.gpsimd.to_reg(SIDX)
            arows = pad + 128
            for acc in accs1 + accs2:
                r0 = 0
                while r0 < arows:
                    rr = min(ZCOL, arows - r0)
                    nc.sync.dma_start(out=acc[r0:r0 + rr, :], in_=zbig[:, :rr])
                    r0 += rr
            nc.sync.dma_start(out=adst1[0:1, pad:pad + TSPAN], in_=zbig[0:1, :TSPAN])
            nc.sync.dma_start(out=adst2[0:1, pad:pad + TSPAN], in_=zbig[0:1, :TSPAN])

            # ---- weight prep ----
            w1sb = sb.tile([d_in, d_hid], FP16)
            nc.gpsimd.dma_start(out=w1sb[:], in_=W1[:])
            w1ext = constp.tile([d_in, d_hid + 2], FP16)
            nc.vector.tensor_copy(w1ext[:, :d_hid], w1sb[:])
            w1t_ps = ps.tile([128, 1024], FP16, tag="scr", name="w1t_ps")[:d_hid, :d_in]
            nc.tensor.transpose(w1t_ps[:], w1sb[:], identh[:d_in, :d_in])
            w1t = sb.tile([d_hid, d_in], FP16)
            nc.vector.tensor_copy(w1t[:], w1t_ps[:])
            a1sb = sb.tile([d_hid, 2], FP16)
            nc.gpsimd.dma_start(out=a1sb[:], in_=a1[:])
            wa_ps = ps.tile([128, 512], FP32, tag="scr", name="wa_ps")[:d_in, :2]
            nc.tensor.matmul(wa_ps[:], w1t[:], a1sb[:], start=True, stop=True)
            nc.vector.tensor_copy(w1ext[:, d_hid:d_hid + 2], wa_ps[:])

            w2sb = sb.tile([d_hid, d_out], FP16)
            nc.gpsimd.dma_start(out=w2sb[:], in_=W2[:])
            w2ext = constp.tile([d_hid, d_out + 2], FP16)
            nc.vector.tensor_copy(w2ext[:, :d_out], w2sb[:])
            w2t_ps = ps.tile([128, 1024], FP16, tag="scr", name="w2t_ps")[:d_out, :d_hid]
            nc.tensor.transpose(w2t_ps[:], w2sb[:], identh[:d_hid, :d_hid])
            w2t = sb.tile([d_out, d_hid], FP16)
            nc.vector.tensor_copy(w2t[:], w2t_ps[:])
            a2sb = sb.tile([d_out, 2], FP16)
            nc.gpsimd.dma_start(out=a2sb[:], in_=a2[:])
            wa2_ps = ps.tile([128, 512], FP32, tag="scr", name="wa2_ps")[:d_hid, :2]
            nc.tensor.matmul(wa2_ps[:], w2t[:], a2sb[:], start=True, stop=True)
            nc.vector.tensor_copy(w2ext[:, d_out:d_out + 2], wa2_ps[:])

            b1row = sb.tile([1, d_hid], FP16)
            nc.gpsimd.dma_start(out=b1row[:], in_=b1[:])
            b1_ps = ps.tile([128, 512], FP32, tag="scr", name="b1_ps")[:, :d_hid]
            nc.tensor.matmul(b1_ps[:], ones_row[:], b1row[:], start=True, stop=True)
            b1rep = constp.tile([128, d_hid], FP32)
            nc.vector.tensor_copy(b1rep[:], b1_ps[:])
            b2row = sb.tile([1, d_out], FP16)
            nc.gpsimd.dma_start(out=b2row[:], in_=b2[:])
            b2_ps = ps.tile([128, 512], FP32, tag="scr", name="b2_ps")[:, :d_out]
            nc.tensor.matmul(b2_ps[:], ones_row[:], b2row[:], start=True, stop=True)
            b2rep = constp.tile([128, d_out], FP32)
            nc.vector.tensor_copy(b2rep[:], b2_ps[:])

            def reduce_part_max(rm, nm):
                """[128, 2] fp32 -> [1, 2] max over partitions (PE transpose)."""
                rt_ps = ps.tile([128, 512], FP32, tag="scr", name=f"rt_ps{nm}")[:2, :128]
                nc.tensor.transpose(rt_ps[:], rm[:], ident[:])
                rt = sb.tile([2, 128], FP32, name=f"rt{nm}")
                nc.vector.tensor_copy(rt[:], rt_ps[:])
                rmx = sb.tile([2, 1], FP32, name=f"rmx{nm}")
                nc.vector.tensor_reduce(rmx[:], rt[:], mybir.AxisListType.X,
                                        op=AluOp.max)
                rmxh = sb.tile([2, 1], FP32, name=f"rmxh{nm}")
                nc.vector.tensor_copy(rmxh[:], rmx[:])
                bm_ps = ps.tile([128, 512], FP32, tag="scr", name=f"bm_ps{nm}")[:1, :2]
                nc.tensor.transpose(bm_ps[:], rmxh[:], ident[:2, :2])
                bout = sb.tile([1, 2], FP32, name=f"bout{nm}")
                nc.vector.tensor_copy(bout[:], bm_ps[:])
                return bout

            def make_negb(bfull, nm):
                bsb = sb.tile([1, 2], FP32, name=f"bsb{nm}")
                nc.sync.dma_start(out=bsb[:], in_=bfull[:])
                bsum = sb.tile([1, 1], FP32, name=f"bsum{nm}")
                nc.vector.tensor_add(bsum[:], bsb[:, 0:1], bsb[:, 1:2])
                bh = sb.tile([1, 1], FP16, name=f"bh{nm}")
                nc.vector.tensor_copy(bh[:], bsum[:])
                nb_ps = ps.tile([128, 512], FP32, tag="scr", name=f"nb_ps{nm}")[:, :1]
                nc.tensor.matmul(nb_ps[:], ones_row[:], bh[:], start=True, stop=True)
                negb = constp.tile([128, 1], FP32, name=f"negb{nm}")
                nc.vector.tensor_scalar(negb[:], nb_ps[:], -1.0, B_MARGIN,
                                        AluOp.mult, AluOp.add)
                return negb

            # ---- phase X ----
            runmax = sb.tile([128, 2], FP32)
            nc.vector.memset(runmax[:], -1e30)
            for blk in range(nblk):
                xb = sb3.tile([128, d_in], FP16, tag="xb")
                nc.gpsimd.dma_start(out=xb[:], in_=x[blk * 128:(blk + 1) * 128, :])
                xt_ps = ps.tile([128, 1024], FP16, tag="scr", name="xt_ps")[:d_in, :128]
                nc.tensor.transpose(xt_ps[:], xb[:], identh[:])
                xt = sb3.tile([d_in, 128], FP16, tag="xt")
                nc.vector.tensor_copy(xt[:], xt_ps[:])
                he_ps = ps.tile([128, 512], FP32, tag="scr", name="he_ps")[:, :d_hid + 2]
                nc.tensor.matmul(he_ps[:], xt[:], w1ext[:], start=True, stop=True)
                row = sb3.tile([128, M1], FP16, tag="row")
                nc.vector.tensor_copy(row[:], he_ps[:, :M1])
                nc.sync.dma_start(out=tab1_sh[blk * 128:(blk + 1) * 128, :M1],
                                  in_=row[:])
                ad = sb3.tile([128, 1], FP16, tag="ad")
                nc.vector.tensor_copy(ad[:], he_ps[:, d_hid + 1:d_hid + 2])
                nc.sync.dma_start(out=adst1[0, blk * 128:(blk + 1) * 128],
                                  in_=ad[:, 0])
                nc.vector.tensor_max(runmax[:], runmax[:], he_ps[:, d_hid:d_hid + 2])

            bmax_sb = reduce_part_max(runmax, "1")
            nc.sync.dma_start(out=bmax_sh[:], in_=bmax_sb[:])
            nc.gpsimd.collective_compute(
                "AllReduce", AluOp.max, replica_groups=rg,
                ins=[bmax_sh[:]], outs=[bmax1[:]])
            nc.gpsimd.collective_compute(
                "AllGather", AluOp.bypass, replica_groups=rg,
                ins=[tab1_sh[:]], outs=[tab1[:]])

            negb1 = make_negb(bmax1, "1")

            def edge_phase(tab, adst_t, accs, negb, d, M):
                GC = GIDX // 16
                SC = SIDX // 16
                for c in range(nch):
                    idxg = sb3.tile([128, NRES * GC], I16, tag="idxg")
                    nc.sync.dma_start(out=idxg[:], in_=idxg_s[c])
                    g = sb3.tile([TEDGE, CH_T, ROW], FP16, tag="g")
                    # the SWDGE descriptor ring caps one gather at 1024
                    # descriptors -> two halves per residue
                    for r in range(NRES):
                        for h in range(2):
                            nc.gpsimd.dma_gather(
                                out_ap=g[:, r * SEG_T + h * 8:
                                         r * SEG_T + (h + 1) * 8, :],
                                in_ap=tab[:, r * ROW:(r + 1) * ROW],
                                idxs_ap=idxg[:, r * GC + h * (GC // 2):
                                             r * GC + (h + 1) * (GC // 2)],
                                num_idxs=GIDX // 2, num_idxs_reg=reg_g,
                                elem_size=ROW, elem_step=NRES * ROW)
                    indb = sb3.tile([TEDGE, CH_T, TSPAN], FP16, tag="ind")
                    nc.sync.dma_start(out=indb[:], in_=ind_s[c])
                    indtb = sb3.tile([TEDGE, SEG_T, TEDGE], FP16, tag="indt")
                    nc.sync.dma_start(out=indtb[:], in_=indt_s[c])
                    aix = sb3.tile([CH_T, 1], I32, tag="aix")
                    nc.sync.dma_start(out=aix[:], in_=adst_i[c])
                    arun = sb3.tile([CH_T, TSPAN], FP16, tag="arun")
                    nc.gpsimd.indirect_dma_start(
                        out=arun[:], out_offset=None, in_=adst_t[:],
                        in_offset=IndirectOffsetOnAxis(ap=aix[:], axis=1))
                    at_ps = ps.tile([128, 1024], FP16, tag="scr", name="at_ps")[:TSPAN, :CH_T]
                    nc.tensor.transpose(at_ps[:], arun[:], identh[:CH_T, :CH_T])
                    at_sb = sb3.tile([TSPAN, CH_T], FP16, tag="at_sb")
                    nc.vector.tensor_copy(at_sb[:], at_ps[:])
                    arep_ps = ps.tile([128, 512], FP32, tag="scr", name="arep_ps")[:, :CH_T]
                    nc.tensor.matmul(arep_ps[:], rrep[:], at_sb[:],
                                     start=True, stop=True)
                    arhs = sb3.tile([128, SEG_T, 4], FP16, tag="arhs")
                    nc.vector.tensor_tensor(
                        arhs[:], arep_ps[:].rearrange("p (b a) -> p b a", a=4),
                        qmask[:].to_broadcast([128, SEG_T, 4]), AluOp.mult)
                    ex_ps = ps.tile([128, 512], FP32, tag="scr", name="ex_ps")[:, :CH_T]
                    for b in range(SEG_T):
                        nc.tensor.matmul(
                            ex_ps[:, 4 * b:4 * (b + 1)],
                            indtb[:, b, :], arhs[:, b, :],
                            start=True, stop=True)
                    tbuf = sb3.tile([TEDGE, CH_T], FP32, tag="tbuf")
                    nc.vector.tensor_add(tbuf[:], g[:, :, d], ex_ps[:])
                    t02 = sb3.tile([TEDGE, CH_T], FP32, tag="t02")
                    nc.vector.tensor_scalar_mul(t02[:], tbuf[:], 0.2)
                    ubuf = sb3.tile([TEDGE, CH_T], FP32, tag="ubuf")
                    nc.vector.tensor_max(ubuf[:], tbuf[:], t02[:])
                    wbuf = sb3.tile([TEDGE, CH_T, 1], FP16, tag="wbuf")
                    nc.scalar.activation(wbuf[:, :, 0], ubuf[:], Act.Exp,
                                         bias=negb[:], scale=1.0)
                    wh = sb3.tile([TEDGE, CH_T, M], FP16, tag="wh")
                    nc.vector.tensor_tensor(
                        wh[:, :, :d], g[:, :, :d],
                        wbuf[:].to_broadcast([TEDGE, CH_T, d]), AluOp.mult)
                    nc.vector.tensor_copy(wh[:, :, d:d + 1], wbuf[:])

                    scgb = sb3.tile([128, NRES * SC], I16, tag="scg")
                    nc.sync.dma_start(out=scgb[:], in_=scg_s[c])
                    for r in range(NRES):
                        pm = psm.tile([128, 512], FP32, tag="msg")
                        for k in range(SEG_T):
                            j = r * SEG_T + k
                            p0 = TSPAN * (k % 3)
                            b = k // 3
                            nc.tensor.matmul(
                                pm[p0:p0 + TSPAN, b * M:(b + 1) * M],
                                indb[:, j, :], wh[:, j, :],
                                start=True, stop=True)
                        cp = sb3.tile([128, 6, M], FP16, tag="cp")
                        nc.vector.tensor_copy(
                            cp[:96, :5, :],
                            pm[:96, :5 * M].rearrange("p (b m) -> p b m", m=M))
                        nc.vector.tensor_copy(cp[0:32, 5, :], pm[0:32, 5 * M:6 * M])
                        nc.gpsimd.dma_scatter_add(
                            out_ap=accs[r][:, :M], in_ap=cp[:, :, :],
                            idxs_ap=scgb[:, r * SC:(r + 1) * SC],
                            num_idxs=SIDX, num_idxs_reg=reg_s,
                            elem_size=M, elem_step=ROW)

            edge_phase(tab1, adst1, accs1, negb1, d_hid, M1)

            # ---- flush layer 1 ----
            runmax2 = sb.tile([128, 2], FP32)
            nc.vector.memset(runmax2[:], -1e30)
            for wk in range(wins):
                h1t = sb3.tile([d_hid, WIN], FP16, tag="h1t")
                for t in range(4):
                    blk = wk * 4 + t
                    sl = slice(blk * 128, (blk + 1) * 128)
                    u01 = sb3.tile([128, M1], FP32, tag="u01")
                    u23 = sb3.tile([128, M1], FP32, tag="u23")
                    a0 = sb3.tile([128, M1], FP16, tag="a0")
                    a1b = sb3.tile([128, M1], FP16, tag="a1b")
                    a2b = sb3.tile([128, M1], FP16, tag="a2b")
                    a3b = sb3.tile([128, M1], FP16, tag="a3b")
                    nc.sync.dma_start(out=a0[:], in_=accs1[0][sl, :M1])
                    nc.sync.dma_start(out=a1b[:], in_=accs1[1][sl, :M1])
                    nc.sync.dma_start(out=a2b[:], in_=accs1[2][sl, :M1])
                    nc.sync.dma_start(out=a3b[:], in_=accs1[3][sl, :M1])
                    nc.vector.tensor_add(u01[:], a0[:], a1b[:])
                    nc.vector.tensor_add(u23[:], a2b[:], a3b[:])
                    ur = sb3.tile([128, M1], FP32, tag="ur")
                    nc.vector.tensor_add(ur[:], u01[:], u23[:])
                    sc_col = sb3.tile([128, 1], FP32, tag="sc_col")
                    nc.vector.tensor_scalar_add(sc_col[:], ur[:, d_hid:d_hid + 1],
                                                1e-16)
                    rec = sb3.tile([128, 1], FP32, tag="rec")
                    nc.vector.reciprocal(rec[:], sc_col[:])
                    z = sb3.tile([128, d_hid], FP32, tag="z")
                    nc.vector.scalar_tensor_tensor(
                        z[:], ur[:, :d_hid], rec[:], b1rep[:],
                        AluOp.mult, AluOp.add)
                    h1r = sb3.tile([128, d_hid], FP16, tag="h1r")
                    nc.scalar.activation(h1r[:], z[:], Act.Relu)
                    h1t_ps = ps.tile([128, 1024], FP16, tag="scr", name="h1t_ps")[:d_hid, :128]
                    nc.tensor.transpose(h1t_ps[:], h1r[:], identh[:])
                    nc.vector.tensor_copy(h1t[:, t * 128:(t + 1) * 128], h1t_ps[:])
                h2e_ps = ps.tile([128, 512], FP32, tag="scr", name="h2e_ps")[:d_out + 2, :]
                nc.tensor.matmul(h2e_ps[:], w2ext[:], h1t[:], start=True, stop=True)
                v = sb3.tile([d_out + 2, WIN], FP32, tag="v")
                nc.vector.tensor_copy(v[:], h2e_ps[:])
                for t in range(4):
                    vt_ps = ps.tile([128, 512], FP32, tag="scr", name="vt_ps")[:, :d_out + 2]
                    nc.tensor.transpose(vt_ps[:], v[:, t * 128:(t + 1) * 128],
                                        ident[:d_out + 2, :d_out + 2])
                    row2 = sb3.tile([128, M2], FP16, tag="row2")
                    nc.vector.tensor_copy(row2[:], vt_ps[:, :M2])
                    r0 = wk * WIN + t * 128
                    nc.sync.dma_start(out=tab2_sh[r0:r0 + 128, :M2], in_=row2[:])
                    ad2 = sb3.tile([128, 1], FP16, tag="ad2")
                    nc.vector.tensor_copy(ad2[:], vt_ps[:, d_out + 1:d_out + 2])
                    nc.sync.dma_start(out=adst2[0, r0:r0 + 128], in_=ad2[:, 0])
                    nc.vector.tensor_max(runmax2[:], runmax2[:],
                                         vt_ps[:, d_out:d_out + 2])

            bmax_sb2 = reduce_part_max(runmax2, "2")
            nc.sync.dma_start(out=bmax_sh[:], in_=bmax_sb2[:])
            nc.gpsimd.collective_compute(
                "AllReduce", AluOp.max, replica_groups=rg,
                ins=[bmax_sh[:]], outs=[bmax2[:]])
            nc.gpsimd.collective_compute(
                "AllGather", AluOp.bypass, replica_groups=rg,
                ins=[tab2_sh[:]], outs=[tab2[:]])
            negb2 = make_negb(bmax2, "2")

            edge_phase(tab2, adst2, accs2, negb2, d_out, M2)

            # ---- flush layer 2: log_softmax ----
            for blk in range(nblk):
                sl = slice(blk * 128, (blk + 1) * 128)
                c0 = sb3.tile([128, M2], FP16, tag="c0")
                c1 = sb3.tile([128, M2], FP16, tag="c1")
                c2 = sb3.tile([128, M2], FP16, tag="c2")
                c3 = sb3.tile([128, M2], FP16, tag="c3")
                nc.sync.dma_start(out=c0[:], in_=accs2[0][sl, :M2])
                nc.sync.dma_start(out=c1[:], in_=accs2[1][sl, :M2])
                nc.sync.dma_start(out=c2[:], in_=accs2[2][sl, :M2])
                nc.sync.dma_start(out=c3[:], in_=accs2[3][sl, :M2])
                s01 = sb3.tile([128, M2], FP32, tag="s01")
                s23 = sb3.tile([128, M2], FP32, tag="s23")
                nc.vector.tensor_add(s01[:], c0[:], c1[:])
                nc.vector.tensor_add(s23[:], c2[:], c3[:])
                ur2 = sb3.tile([128, M2], FP32, tag="ur2")
                nc.vector.tensor_add(ur2[:], s01[:], s23[:])
                sc2c = sb3.tile([128, 1], FP32, tag="sc2c")
                nc.vector.tensor_scalar_add(sc2c[:], ur2[:, d_out:d_out + 1], 1e-16)
                rec2 = sb3.tile([128, 1], FP32, tag="rec2")
                nc.vector.reciprocal(rec2[:], sc2c[:])
                z2 = sb3.tile([128, d_out], FP32, tag="z2")
                nc.vector.scalar_tensor_tensor(
                    z2[:], ur2[:, :d_out], rec2[:], b2rep[:],
                    AluOp.mult, AluOp.add)
                mx = sb3.tile([128, 1], FP32, tag="mx")
                nc.vector.tensor_reduce(mx[:], z2[:], mybir.AxisListType.X,
                                        op=AluOp.max)
                nmx = sb3.tile([128, 1], FP32, tag="nmx")
                nc.vector.tensor_scalar_mul(nmx[:], mx[:], -1.0)
                es = sb3.tile([128, d_out], FP32, tag="es")
                sume = sb3.tile([128, 1], FP32, tag="sume")
                nc.scalar.activation(es[:], z2[:], Act.Exp, bias=nmx[:],
                                     scale=1.0, accum_out=sume[:])
                lns = sb3.tile([128, 1], FP32, tag="lns")
                nc.scalar.activation(lns[:], sume[:], Act.Ln)
                tot = sb3.tile([128, 1], FP32, tag="tot")
                nc.vector.tensor_add(tot[:], mx[:], lns[:])
                fin = sb3.tile([128, d_out], FP32, tag="fin")
                nc.vector.scalar_tensor_tensor(
                    fin[:], z2[:], tot[:], tot[:].to_broadcast([128, d_out]),
                    AluOp.subtract, AluOp.bypass)
                nc.sync.dma_start(out=out[sl, :], in_=fin[:])

    return nc


# ---------------------------------------------------------------------------
# Entry point
# ---------------------------------------------------------------------------

_CACHE = {}


class SpmdRunner:
    """Build the jitted 8-core executable once; reuse across calls."""

    def __init__(self, nc, n_cores):
        import jax
        from jax.sharding import Mesh, PartitionSpec
        from jax.experimental.shard_map import shard_map
        from concourse.bass2jax import (_bass_exec_p, install_neuronx_cc_hook,
                                        partition_id_tensor)
        install_neuronx_cc_hook()
        self.nc = nc
        self.n_cores = n_cores
        partition_name = nc.partition_id_tensor.name if nc.partition_id_tensor else None
        in_names, out_names, out_avals, zero_outs = [], [], [], []
        for alloc in nc.m.functions[0].allocations:
            if not isinstance(alloc, mybir.MemoryLocationSet):
                continue
            name = alloc.memorylocations[0].name
            if alloc.kind == "ExternalInput":
                if name != partition_name and name != (nc.dbg_addr.name if nc.dbg_addr else None):
                    in_names.append(name)
            elif alloc.kind == "ExternalOutput":
                out_names.append(name)
                shape = tuple(alloc.tensor_shape)
                dtype = mybir.dt.np(alloc.dtype)
                out_avals.append(jax.core.ShapedArray(shape, dtype))
                zero_outs.append(np.zeros(shape, dtype))
        self.in_names, self.out_names = in_names, out_names
        self.out_avals, self.zero_outs = out_avals, zero_outs
        n_params, n_outs = len(in_names), len(out_avals)
        all_in = in_names + out_names + ([partition_name] if partition_name else [])
        if nc.dbg_addr is not None:
            all_in.append(nc.dbg_addr.name)
        self.n_params = n_params

        def _body(*args):
            operands = list(args)
            if nc.dbg_addr is not None:
                operands.append(jax.numpy.zeros((1, 2), jax.numpy.uint32))
            if partition_name is not None:
                operands.append(partition_id_tensor())
            return tuple(_bass_exec_p.bind(
                *operands, out_avals=tuple(out_avals), in_names=tuple(all_in),
                out_names=tuple(out_names), lowering_input_output_aliases=(),
                sim_require_finite=True, sim_require_nnan=True, nc=nc))

        devices = jax.devices()[:n_cores]
        mesh = Mesh(np.asarray(devices), ("core",))
        self._mesh = mesh
        # output-init buffers ride as ordinary (non-donated) parameters, so
        # the SAME device-resident zero arrays can be reused every call —
        # no host->device push of the output size per call. The kernel
        # fully writes every output row it returns, so the zeros content
        # is never observed stale.
        in_specs = (PartitionSpec("core"),) * (n_params + n_outs)
        out_specs = (PartitionSpec("core"),) * len(out_names)
        self._jax = jax
        self._sharded = jax.jit(
            shard_map(_body, mesh=mesh, in_specs=in_specs, out_specs=out_specs,
                      check_rep=False),
            keep_unused=True)

    def prep_inputs(self, in_maps, device_resident=True):
        per_core = [[np.asarray(m[n]) for n in self.in_names] for m in in_maps]
        concat = [np.concatenate([per_core[c][i] for c in range(self.n_cores)], 0)
                  for i in range(self.n_params)]
        concat += [np.zeros((self.n_cores * z.shape[0], *z.shape[1:]), z.dtype)
                   for z in self.zero_outs]
        if not device_resident:
            return concat
        # push once; repeated runs then skip the host->device transfer
        jax = self._jax
        from jax.sharding import NamedSharding, PartitionSpec
        sh = NamedSharding(self._mesh, PartitionSpec("core"))
        return [jax.device_put(a, sh) for a in concat]

    def run(self, concat_in):
        out = self._sharded(*concat_in)
        self._jax.block_until_ready(out)
        return out

    def split_outputs(self, out_arrs):
        return [
            {n: np.asarray(out_arrs[i]).reshape(self.n_cores, *self.out_avals[i].shape)[c]
             for i, n in enumerate(self.out_names)}
            for c in range(self.n_cores)
        ]


def make_in_maps(x, W1, a_src1, a_dst1, b1, W2, a_src2, a_dst2, b2,
                 data, npc, pad, n):
    d_in = x.shape[1]
    xpad = np.zeros((CORES * pad, d_in), np.float32)
    for c in range(CORES):
        lo, hi = c * npc, min((c + 1) * npc, n)
        xpad[c * pad:c * pad + (hi - lo)] = x[lo:hi]
    in_maps = []
    for c in range(CORES):
        d = data[c]
        in_maps.append({
            "x": xpad[c * pad:(c + 1) * pad],
            "W1": np.asarray(W1, np.float32),
            "a1": np.stack([np.asarray(a_src1), np.asarray(a_dst1)], 1).astype(np.float32),
            "b1": np.asarray(b1, np.float32)[None, :],
            "W2": np.asarray(W2, np.float32),
            "a2": np.stack([np.asarray(a_src2), np.asarray(a_dst2)], 1).astype(np.float32),
            "b2": np.asarray(b2, np.float32)[None, :],
            "idxg_s": d["idxg"], "ind_s": d["ind"], "indt_s": d["indt"],
            "adst_i": d["adst_idx"], "scg_s": d["scg"],
        })
    return in_maps


def kernel(x, edges_index, W1, a_src1, a_dst1, b1, W2, a_src2, a_dst2, b2):
    x = np.asarray(x, np.float32)
    edges_index = np.asarray(edges_index)
    n, d_in = x.shape
    d_hid = np.asarray(W1).shape[1]
    d_out = np.asarray(W2).shape[1]

    data, nch, npc, pad = preprocess(edges_index, n)

    key = (nch, pad, npc, d_in, d_hid, d_out)
    if key not in _CACHE:
        nc = build_nc(*key)
        split_excess_waits(nc)
        mybir.codegen_inst_isa_subclasses(nc)  # .instr for the library reload
        _CACHE[key] = SpmdRunner(nc, CORES)
    r = _CACHE[key]

    in_maps = make_in_maps(x, W1, a_src1, a_dst1, b1, W2, a_src2, a_dst2, b2,
                           data, npc, pad, n)
    ci = r.prep_inputs(in_maps)
    outs = r.split_outputs(r.run(ci))
    res = np.concatenate([outs[c]["out"][:npc] for c in range(CORES)], 0)[:n]
    return res.astype(np.float32)
